# revision 1
# baseline (speedup 1.0000x reference)
"""GAT-style kernel for Trainium2, 8 NeuronCores.

Math (per head, d = nhid):
    h  = xf @ W.T + b                  (N, d)
    h1 = h / max(||h||_row, eps)       row L2 normalize
    e  = h1 @ h1.T                     (N, N)  -- never materialized
    att = e / ||e||_col                column L2 normalize
    out = act(att @ h1)

Collapse: with G = h1.T @ h1 (d x d),
    ||e||_col[j]^2 = h1_j.T G h1_j     (row-wise quadratic form)
    att @ h1 = h1 @ M,  M = h1.T @ (h1 / cn)   (d x d)
so the N x N attention matrix is never formed and the whole computation
is O(N d^2).

Two modes:
  * "rep"  (default): every core gets the full input and computes the
    full output; no collectives.  On this stack AllReduce costs ~700us
    per call, far more than the whole collapsed computation, so
    replication wins despite 8x redundant compute.
  * "shard": row-shard N across the 8 cores, 4 small AllReduces
    (G, M for the hidden heads batched; G_o, M_o for the output head).

Measured (device For_i loop amplification, axon/fake_nrt stack):
  rep mode ~0.23 ms per invocation, max rel err ~5e-6 vs the fp32 jax
  reference.  The stack bills ~0.3us per producer->consumer sync edge
  and almost nothing per element, so emission style is chosen per
  phase by measurement: stage 1a uses whole-tensor mega-ops with
  batched PSUM drains; stage 1b, the elu phase and the output head use
  per-tile/per-group pipelined chains rotating through PSUM slots
  (bufs=2-4), which overlap better across tiles there.
"""

import sys

for _p in ("/opt/trn_rl_repo", "/root/.axon_site/_ro/trn_rl_repo"):
    if _p not in sys.path:
        sys.path.append(_p)

import numpy as np

N_CORES = 8
N = 4096
NLOC = N // N_CORES          # 512 rows per core in shard mode
NFEAT = 128
NHID = 64
NCLASS = 16
EPS = 1e-12

_prog_cache = {}
PHASES = {1, 2, 3, 4, 5, 6}  # surgical-profiling switch (timing experiments)


def _patch_tile_drain():
    """Walrus in this container rejects Tile's tail drain (too many sync
    waits on one instruction).  Split it into one-wait-per-drain."""
    import concourse.tile as tile
    from concourse.vector_clock import ScopedClock, VectorClock

    if getattr(tile.TileContext, "_drain_split_patched", False):
        return

    def _drain_and_barrier(self, tick_clock, wait_clock):
        nc = self.nc
        gvc = tick_clock.global_clock  # VectorClock
        n = len(gvc)
        for proc in range(n):
            t = gvc[proc]
            if t > 0:
                sub = VectorClock([t if i == proc else 0 for i in range(n)])
                d = nc.sync.drain()
                wait_clock.add_sem_waits(d.ins, ScopedClock({None: sub}))
        nc.all_engine_barrier()
        assert self.sems is not None
        popped = nc._tile_sem_poison_stack.pop()
        assert popped is self._sem_poison
        nc.clear_and_free_semaphores(list(self.sems.allocated().values()))
        nc.all_engine_barrier()

    tile.TileContext._drain_and_barrier = _drain_and_barrier
    tile.TileContext._drain_split_patched = True


def _split_multi_waits(nc):
    """This container's walrus allows only one sync-wait per instruction.
    Hoist extra waits onto standalone same-engine NoOps."""
    import concourse.mybir as mybir

    n_new = 0
    for blk in nc.main_func.blocks:
        out = []
        changed = False
        for inst in blk.instructions:
            si = inst.sync_info
            waits = list(si.on_wait) if (si and si.on_wait) else []
            if len(waits) > 1:
                changed = True
                for w in waits[:-1]:
                    nop = mybir.InstNoOp(name=f"{inst.name}-xw{n_new}", ins=[], outs=[])
                    n_new += 1
                    nop.engine = inst.engine
                    nop.sync_info = mybir.SyncInfo(on_wait=[w], on_update=[])
                    out.append(nop)
                si.on_wait = [waits[-1]]
                inst.sync_info = si
            out.append(inst)
        if changed:
            blk.instructions = out


def _norm_scalars(nc, pool, q, name_tag, width=1):
    """q (128,w) sums of squares -> rn (128,w) = 1/max(sqrt(q), EPS).

    Computed as sqrt(1/q).  The reference's max(norm, EPS=1e-12) guard is
    dropped: q is a sum of >= 16 squares of O(1) activations, orders of
    magnitude above EPS^2 for any realistic input."""
    import concourse.mybir as mybir

    f32 = mybir.dt.float32
    qi = pool.tile([128, width], f32, tag=f"qi_{name_tag}", name=f"qi_{name_tag}")
    nc.vector.reciprocal(qi[:], q[:])
    rn = pool.tile([128, width], f32, tag=f"rn_{name_tag}", name=f"rn_{name_tag}")
    nc.scalar.sqrt(rn[:], qi[:])
    return rn


def _allreduce(nc, dram_pool, sbuf_in, shape, tag):
    """AllReduce sbuf_in (shape) across all cores via DRAM bounce buffers."""
    import concourse.mybir as mybir

    bnc_in = dram_pool.tile(shape, mybir.dt.float32, name=f"arin_{tag}")
    bnc_out = dram_pool.tile(
        shape, mybir.dt.float32, addr_space="Shared", name=f"arout_{tag}"
    )
    nc.sync.dma_start(out=bnc_in[:], in_=sbuf_in[:])
    nc.gpsimd.collective_compute(
        "AllReduce",
        mybir.AluOpType.add,
        replica_groups=[list(range(N_CORES))],
        ins=[bnc_in.opt()],
        outs=[bnc_out.opt()],
    )
    return bnc_out


def _emit_body(nc, tc, ctx, tensors, rep, nloc, use_collectives, with_bias):
    """v3: optimized for this stack's cost model, which is dominated by
    per-instruction sync overhead (~0.3us per producer->consumer edge) with
    nearly free per-element throughput.  Tactics:
      * mega-ops: one DVE/ACT instruction over all row tiles at once
      * batched PSUM drains: several matmul/transpose results per copy
      * alternate PSUM-drain copies between DVE and ACT streams
    """
    import concourse.bass as bass
    import concourse.mybir as mybir
    from concourse.bass import ts
    from concourse.masks import make_identity

    f32 = mybir.dt.float32
    r = rep
    nt = nloc // 128        # row tiles (32 in rep mode)
    nch = nloc // 512       # 512-wide column chunks in transposed layout
    D3 = 3 * NHID

    x_d = tensors["xloc"]
    w123t_d = tensors["w123t"]
    b123_d = tensors["b123"]
    wot_hi_d = tensors["wot_hi"]
    wot_lo_d = tensors["wot_lo"]
    bo_d = tensors["bo"]
    out_d = tensors["outt"]

    const = ctx.enter_context(tc.tile_pool(name=f"const{r}", bufs=1))
    work = ctx.enter_context(tc.tile_pool(name=f"work{r}", bufs=1))
    small = ctx.enter_context(tc.tile_pool(name=f"small{r}", bufs=2))
    # PSUM: wide (4 banks) + mm16 (2) + acc01 (1) + accsm (1) = 8 banks
    psum = ctx.enter_context(tc.tile_pool(name=f"psum{r}", bufs=1, space="PSUM"))
    dram = ctx.enter_context(tc.tile_pool(name=f"dram{r}", bufs=1, space="DRAM"))

    def ps_tile(shape, tag, name, bufs=1):
        return psum.tile(
            shape, mybir.dt.float32, tag=tag, name=f"{name}_{r}", bufs=bufs
        )

    def bcast_free(ap, inner):
        """Broadcast an AP with a trailing stride-0 inner dim."""
        return bass.AP(tensor=ap.tensor, offset=ap.offset, ap=[*ap.ap, [0, inner]])

    copy_flip = [0]

    def drain_copy(dst, src):
        """PSUM->SBUF copy, alternating DVE/ACT to split the sync load."""
        if copy_flip[0] & 1:
            nc.scalar.copy(dst, src)
        else:
            nc.vector.tensor_copy(dst, src)
        copy_flip[0] += 1

    # ---- constants / inputs ----
    w123t_sb = const.tile([128, D3], f32, name=f"w123t{r}")
    nc.sync.dma_start(out=w123t_sb[:], in_=w123t_d[:])
    wot_hi_sb = const.tile([128, NCLASS], f32, name=f"wot_hi{r}")
    nc.sync.dma_start(out=wot_hi_sb[:], in_=wot_hi_d[:])
    wot_lo_sb = const.tile([64, NCLASS], f32, name=f"wot_lo{r}")
    nc.sync.dma_start(out=wot_lo_sb[:], in_=wot_lo_d[:])
    if with_bias:
        b123_sb = const.tile([1, D3], f32, name=f"b123{r}")
        nc.sync.dma_start(out=b123_sb[:], in_=b123_d[:])
        bo_sb = const.tile([1, NCLASS], f32, name=f"bo{r}")
        nc.sync.dma_start(out=bo_sb[:], in_=bo_d[:])
        ones_row = const.tile([1, 128], f32, name=f"ones{r}")
        nc.vector.memset(ones_row[:], 1.0)
    id128 = const.tile([128, 128], f32, name=f"id128_{r}")
    make_identity(nc, id128[:])

    # persistent tensors; heads 0,1 ride together on 128 partitions
    h1a = const.tile([128, nt, D3], f32, name=f"h1a_{r}")
    h1t01 = const.tile([128, nloc], f32, name=f"h1t01_{r}")
    h1t2 = const.tile([NHID, nloc], f32, name=f"h1t2_{r}")
    hc_hi = const.tile([128, nloc], f32, name=f"hc_hi{r}")
    hc_lo = const.tile([64, nloc], f32, name=f"hc_lo{r}")

    # "big" slot: sequentially reused 24KB/partition scratch
    # (x -> squares -> ta -> z), one SBUF allocation
    def big_tile(shape, name):
        return work.tile(shape, f32, tag="big", name=f"{name}_{r}")

    # =================== stage 1a ===================
    x_sb = big_tile([128, nloc], f"x_sb")
    nc.sync.dma_start(out=x_sb[:], in_=x_d[:])
    if 1 in PHASES:
        # h = x_t.T @ W.T (+b): two tiles per PSUM batch, one drain each
        for i in range(nt // 2):
            ha_ps = ps_tile([128, 2, D3], "wide", f"ha{i}", bufs=4)
            for j in range(2):
                t = 2 * i + j
                nc.tensor.matmul(
                    ha_ps[:, j, :], x_sb[:, ts(t, 128)], w123t_sb[:],
                    start=True, stop=not with_bias,
                )
                if with_bias:
                    nc.tensor.matmul(
                        ha_ps[:, j, :], ones_row[:], b123_sb[:],
                        start=False, stop=True,
                    )
            drain_copy(h1a[:, ts(i, 2), :], ha_ps[:])
        # row norms: one mega-op per pipeline step
        scr = big_tile([128, nt * D3], "scr1")
        nc.vector.tensor_mul(
            scr[:], h1a[:].rearrange("p a b -> p (a b)"),
            h1a[:].rearrange("p a b -> p (a b)"),
        )
        q = small.tile([128, nt * 3], f32, tag="q", name=f"q_{r}")
        nc.vector.reduce_sum(
            q[:],
            scr[:].rearrange("p (t k d) -> p (t k) d", k=3, d=NHID),
            axis=mybir.AxisListType.X,
        )
        qi = small.tile([128, nt * 3], f32, tag="qi", name=f"qi_{r}")
        nc.vector.reciprocal(qi[:], q[:])
        rn = small.tile([128, nt * 3], f32, tag="rn", name=f"rn_{r}")
        nc.scalar.sqrt(rn[:], qi[:])
        nc.vector.tensor_mul(
            h1a[:].rearrange("p t (k d) -> p (t k) d", k=3),
            h1a[:].rearrange("p t (k d) -> p (t k) d", k=3),
            bcast_free(rn[:], NHID),
        )
        # transposes into (d, n) layout, 4 per PSUM batch
        for i in range(nt // 4):
            trb = ps_tile([128, 4, 128], "wide", f"trb{i}", bufs=4)
            for j in range(4):
                nc.tensor.transpose(
                    trb[:, j, :], h1a[:, 4 * i + j, 0:128], id128[:]
                )
            drain_copy(
                h1t01[:, ts(i, 512)].rearrange("p (a b) -> p a b", a=4), trb[:]
            )
            trb2 = ps_tile([NHID, 4, 128], "wide", f"trb2_{i}", bufs=4)
            for j in range(4):
                nc.tensor.transpose(
                    trb2[:, j, :], h1a[:, 4 * i + j, 128:192], id128[:]
                )
            drain_copy(
                h1t2[:, ts(i, 512)].rearrange("p (a b) -> p a b", a=4), trb2[:]
            )
        # Gram accumulation
        g01_ps = ps_tile([128, 128], "acc01", "g01")
        g2_ps = ps_tile([NHID, NHID], "accsm", "g2")
        for t in range(nt):
            nc.tensor.matmul(
                g01_ps[:], h1a[:, t, 0:128], h1a[:, t, 0:128],
                start=(t == 0), stop=(t == nt - 1),
            )
            nc.tensor.matmul(
                g2_ps[:], h1a[:, t, 128:192], h1a[:, t, 128:192],
                start=(t == 0), stop=(t == nt - 1),
            )

    # ---- global G as block-diag gblk (heads 0,1) + g2 ----
    gblk = const.tile([128, 128], f32, name=f"gblk{r}")
    g2_sb = const.tile([NHID, NHID], f32, name=f"g2sb{r}")
    if 1 in PHASES:
        if use_collectives:
            gcat = const.tile([NHID, D3], f32, name=f"gcat{r}")
            nc.vector.tensor_copy(gcat[:, 0:NHID], g01_ps[0:NHID, 0:NHID])
            gtmp = const.tile([128, NHID], f32, name=f"gtmp{r}")
            nc.vector.tensor_copy(gtmp[NHID:128, :], g01_ps[NHID:128, NHID:128])
            nc.sync.dma_start(out=gcat[:, NHID : 2 * NHID], in_=gtmp[NHID:128, :])
            nc.vector.tensor_copy(gcat[:, 2 * NHID :], g2_ps[:])
            g_out = _allreduce(nc, dram, gcat, [NHID, D3], f"g_{r}")
            g_glob = const.tile([NHID, D3], f32, name=f"g_glob{r}")
            nc.sync.dma_start(out=g_glob[:], in_=g_out[:])
            nc.vector.memset(gblk[:], 0.0)
            nc.vector.tensor_copy(gblk[0:NHID, 0:NHID], g_glob[:, 0:NHID])
            nc.sync.dma_start(
                out=gblk[NHID:128, NHID:128], in_=g_glob[:, NHID : 2 * NHID]
            )
            nc.vector.tensor_copy(g2_sb[:], g_glob[:, 2 * NHID :])
        else:
            nc.vector.memset(gblk[:], 0.0)
            nc.vector.tensor_copy(gblk[0:NHID, 0:NHID], g01_ps[0:NHID, 0:NHID])
            nc.vector.tensor_copy(gblk[NHID:128, NHID:128], g01_ps[NHID:128, NHID:128])
            nc.scalar.copy(g2_sb[:], g2_ps[:])

    # =================== stage 1b ===================
    if 2 in PHASES:
        # t = h1 @ G then p-row product, per-tile pipelined (the per-tile
        # DVE mul frees the PSUM slot; tiles overlap via slot rotation)
        # software-pipelined emission: the DVE product for tile t-1 is
        # emitted after tile t's matmuls, so its producer has finished by
        # the time DVE reaches it (waits satisfied on arrival)
        scrs = []
        ta_tiles = []

        def emit_prod(u):
            scr = work.tile([128, D3], f32, tag="scr2", name=f"scr2_{u}_{r}", bufs=4)
            nc.vector.tensor_mul(scr[:], ta_tiles[u][:], h1a[:, u, :])
            scrs.append(scr)

        for t in range(nt):
            ta_ps = ps_tile([128, D3], "wide", f"ta{t}", bufs=4)
            nc.tensor.matmul(
                ta_ps[:, 0:128], h1t01[:, ts(t, 128)], gblk[:],
                start=True, stop=True,
            )
            nc.tensor.matmul(
                ta_ps[:, 128:192], h1t2[:, ts(t, 128)], g2_sb[:],
                start=True, stop=True,
            )
            ta_tiles.append(ta_ps)
            if t >= 1:
                emit_prod(t - 1)
        emit_prod(nt - 1)
        p3s = []
        for t in range(nt):
            p3 = small.tile([128, 3], f32, tag="p3", name=f"p3_{t}_{r}", bufs=8)
            nc.vector.reduce_sum(
                p3[:],
                scrs[t][:].rearrange("p (k d) -> p k d", k=3),
                axis=mybir.AxisListType.X,
            )
            p3s.append(p3)
        pis = []
        for t in range(nt):
            pi = small.tile([128, 3], f32, tag="pi", name=f"pi_{t}_{r}", bufs=8)
            nc.vector.reciprocal(pi[:], p3s[t][:])
            pis.append(pi)
        icns = []
        for t in range(nt):
            icn = small.tile([128, 3], f32, tag="icn", name=f"icn_{t}_{r}", bufs=8)
            nc.scalar.sqrt(icn[:], pis[t][:])
            icns.append(icn)
        h1ss = []
        for t in range(nt):
            h1s = work.tile([128, D3], f32, tag="h1s", name=f"h1s_{t}_{r}", bufs=4)
            nc.vector.tensor_mul(
                h1s[:].rearrange("p (k d) -> p k d", k=3),
                h1a[:, t, :].rearrange("p (k d) -> p k d", k=3),
                bcast_free(icns[t][:], NHID),
            )
            h1ss.append(h1s)
        m01_ps = ps_tile([128, 128], "acc01", "m01")
        m2_ps = ps_tile([NHID, NHID], "accsm", "m2")
        for t in range(nt):
            nc.tensor.matmul(
                m01_ps[:], h1a[:, t, 0:128], h1ss[t][:, 0:128],
                start=(t == 0), stop=(t == nt - 1),
            )
            nc.tensor.matmul(
                m2_ps[:], h1a[:, t, 128:192], h1ss[t][:, 128:192],
                start=(t == 0), stop=(t == nt - 1),
            )

    # ---- global M as block-diag mblk (heads 0,1) + m2 ----
    mblk = const.tile([128, 128], f32, name=f"mblk{r}")
    m2_sb = const.tile([NHID, NHID], f32, name=f"m2sb{r}")
    if 2 in PHASES:
        if use_collectives:
            mcat = const.tile([NHID, D3], f32, name=f"mcat{r}")
            nc.vector.tensor_copy(mcat[:, 0:NHID], m01_ps[0:NHID, 0:NHID])
            mtmp = const.tile([128, NHID], f32, name=f"mtmp{r}")
            nc.vector.tensor_copy(mtmp[NHID:128, :], m01_ps[NHID:128, NHID:128])
            nc.sync.dma_start(out=mcat[:, NHID : 2 * NHID], in_=mtmp[NHID:128, :])
            nc.vector.tensor_copy(mcat[:, 2 * NHID :], m2_ps[:])
            m_out = _allreduce(nc, dram, mcat, [NHID, D3], f"m_{r}")
            m_glob = const.tile([NHID, D3], f32, name=f"m_glob{r}")
            nc.sync.dma_start(out=m_glob[:], in_=m_out[:])
            nc.vector.memset(mblk[:], 0.0)
            nc.vector.tensor_copy(mblk[0:NHID, 0:NHID], m_glob[:, 0:NHID])
            nc.sync.dma_start(
                out=mblk[NHID:128, NHID:128], in_=m_glob[:, NHID : 2 * NHID]
            )
            nc.vector.tensor_copy(m2_sb[:], m_glob[:, 2 * NHID :])
        else:
            nc.vector.memset(mblk[:], 0.0)
            nc.vector.tensor_copy(mblk[0:NHID, 0:NHID], m01_ps[0:NHID, 0:NHID])
            nc.vector.tensor_copy(mblk[NHID:128, NHID:128], m01_ps[NHID:128, NHID:128])
            nc.scalar.copy(m2_sb[:], m2_ps[:])

    # =================== z = h1 @ M, elu -> hc ===================
    if 3 in PHASES:
        # elu(z) = exp(min(z,0)) + (max(z,0) - 1), per 512-chunk from PSUM
        for c in range(nch):
            zt_ps = ps_tile([128, 512], "wide", f"zt01_{c}", bufs=4)
            nc.tensor.matmul(
                zt_ps[:], mblk[:], h1t01[:, ts(c, 512)], start=True, stop=True
            )
            zt2_ps = ps_tile([NHID, 512], "wide", f"zt2_{c}", bufs=4)
            nc.tensor.matmul(
                zt2_ps[:], m2_sb[:], h1t2[:, ts(c, 512)], start=True, stop=True
            )
            halves = (
                (zt_ps, 128, hc_hi[:, ts(c, 512)], "hi"),
                (zt2_ps, NHID, hc_lo[:, ts(c, 512)], "lo"),
            )
            emins, emaxs = {}, {}
            for zt, parts, dst, nm in halves:
                e_min = work.tile(
                    [parts, 512], f32, tag=f"emin{nm}",
                    name=f"emin{nm}_{c}_{r}", bufs=2,
                )
                nc.vector.tensor_scalar_min(e_min[:], zt[:], 0.0)
                e_max = work.tile(
                    [parts, 512], f32, tag=f"emax{nm}",
                    name=f"emax{nm}_{c}_{r}", bufs=2,
                )
                nc.vector.tensor_scalar(
                    out=e_max[:], in0=zt[:], scalar1=0.0, scalar2=-1.0,
                    op0=mybir.AluOpType.max, op1=mybir.AluOpType.add,
                )
                emins[nm], emaxs[nm] = e_min, e_max
            for zt, parts, dst, nm in halves:
                nc.scalar.activation(
                    emins[nm][:], emins[nm][:], mybir.ActivationFunctionType.Exp
                )
            for zt, parts, dst, nm in halves:
                nc.vector.tensor_add(dst, emins[nm][:], emaxs[nm][:])

    # =================== stage 2 (output head, d=16) ===================
    h1oa = const.tile([128, nt, NCLASS], f32, name=f"h1oa{r}")
    h1ot = const.tile([NCLASS, nloc], f32, name=f"h1ot{r}")
    if 4 in PHASES:
        # ho = hc @ Wo.T (+bo): 4-tile groups, per-group norm chains
        for g in range(nt // 4):
            hoa_ps = ps_tile([128, 4, NCLASS], "mm16", f"hoa_{g}", bufs=2)
            for j in range(4):
                t = 4 * g + j
                nc.tensor.matmul(
                    hoa_ps[:, j, :], hc_hi[:, ts(t, 128)], wot_hi_sb[:],
                    start=True, stop=False,
                )
                nc.tensor.matmul(
                    hoa_ps[:, j, :], hc_lo[:, ts(t, 128)], wot_lo_sb[:],
                    start=False, stop=not with_bias,
                )
                if with_bias:
                    nc.tensor.matmul(
                        hoa_ps[:, j, :], ones_row[:], bo_sb[:],
                        start=False, stop=True,
                    )
            scro = work.tile(
                [128, 4 * NCLASS], f32, tag="scro", name=f"scro_{g}_{r}", bufs=4
            )
            nc.scalar.activation(
                scro[:], hoa_ps[:], mybir.ActivationFunctionType.Square
            )
            q4 = small.tile([128, 4], f32, tag="qod", name=f"qo_{g}_{r}", bufs=8)
            nc.vector.reduce_sum(
                q4[:],
                scro[:].rearrange("p (j d) -> p j d", j=4),
                axis=mybir.AxisListType.X,
            )
            q4i = small.tile([128, 4], f32, tag="qoi", name=f"qoi_{g}_{r}", bufs=8)
            nc.vector.reciprocal(q4i[:], q4[:])
            rno4 = small.tile([128, 4], f32, tag="rno", name=f"rno_{g}_{r}", bufs=8)
            nc.scalar.sqrt(rno4[:], q4i[:])
            nc.vector.tensor_mul(
                h1oa[:, ts(g, 4), :],
                hoa_ps[:],
                bcast_free(rno4[:], NCLASS),
            )
        # transposes, 4 per PSUM batch
        for i in range(nt // 4):
            trob = ps_tile([NCLASS, 4, 128], "wide", f"trob{i}", bufs=4)
            for j in range(4):
                nc.tensor.transpose(
                    trob[:, j, :], h1oa[:, 4 * i + j, :], id128[:]
                )
            drain_copy(
                h1ot[:, ts(i, 512)].rearrange("p (a b) -> p a b", a=4), trob[:]
            )
        go_ps = ps_tile([NCLASS, NCLASS], "accsm", "go_ps")
        for t in range(nt):
            nc.tensor.matmul(
                go_ps[:], h1oa[:, t, :], h1oa[:, t, :],
                start=(t == 0), stop=(t == nt - 1),
            )
    go_sb = const.tile([NCLASS, NCLASS], f32, name=f"go_sb{r}")
    if 4 in PHASES:
        nc.vector.tensor_copy(go_sb[:], go_ps[:])

    if use_collectives:
        go_out = _allreduce(nc, dram, go_sb, [NCLASS, NCLASS], f"go_{r}")
        go_glob = const.tile([NCLASS, NCLASS], f32, name=f"go_glob{r}")
        nc.sync.dma_start(out=go_glob[:], in_=go_out[:])
    else:
        go_glob = go_sb

    if 5 in PHASES:
        mo_ps = ps_tile([NCLASS, NCLASS], "accsm", "mo_ps")
        for g in range(nt // 4):
            to_ps = ps_tile([128, 4, NCLASS], "mm16", f"to_{g}", bufs=2)
            for j in range(4):
                t = 4 * g + j
                nc.tensor.matmul(
                    to_ps[:, j, :], h1ot[:, ts(t, 128)], go_glob[:],
                    start=True, stop=True,
                )
            scro2 = work.tile(
                [128, 4 * NCLASS], f32, tag="scro2", name=f"scro2_{g}_{r}", bufs=4
            )
            nc.vector.tensor_mul(
                scro2[:].rearrange("p (j d) -> p j d", j=4),
                to_ps[:],
                h1oa[:, ts(g, 4), :],
            )
            p4 = small.tile([128, 4], f32, tag="qod", name=f"po_{g}_{r}", bufs=8)
            nc.vector.reduce_sum(
                p4[:],
                scro2[:].rearrange("p (j d) -> p j d", j=4),
                axis=mybir.AxisListType.X,
            )
            p4i = small.tile([128, 4], f32, tag="qoi", name=f"poi_{g}_{r}", bufs=8)
            nc.vector.reciprocal(p4i[:], p4[:])
            icno4 = small.tile([128, 4], f32, tag="rno", name=f"icno_{g}_{r}", bufs=8)
            nc.scalar.sqrt(icno4[:], p4i[:])
            h1so = work.tile(
                [128, 4 * NCLASS], f32, tag="h1so", name=f"h1so_{g}_{r}", bufs=4
            )
            nc.vector.tensor_mul(
                h1so[:].rearrange("p (j d) -> p j d", j=4),
                h1oa[:, ts(g, 4), :],
                bcast_free(icno4[:], NCLASS),
            )
            for j in range(4):
                t = 4 * g + j
                nc.tensor.matmul(
                    mo_ps[:], h1oa[:, t, :], h1so[:, ts(j, NCLASS)],
                    start=(t == 0), stop=(t == nt - 1),
                )
    mo_sb = const.tile([NCLASS, NCLASS], f32, name=f"mo_sb{r}")
    if 5 in PHASES:
        nc.vector.tensor_copy(mo_sb[:], mo_ps[:])

    if use_collectives:
        mo_out = _allreduce(nc, dram, mo_sb, [NCLASS, NCLASS], f"mo_{r}")
        mo_glob = const.tile([NCLASS, NCLASS], f32, name=f"mo_glob{r}")
        nc.sync.dma_start(out=mo_glob[:], in_=mo_out[:])
    else:
        mo_glob = mo_sb

    # ---- final: out = (h1o @ Mo).T = Mo.T @ h1o.T ----
    fot_sb = const.tile([NCLASS, nloc], f32, name=f"fot_sb{r}")
    for c in range(nch) if 6 in PHASES else []:
        fot_ps = ps_tile([NCLASS, 512], "wide", f"fot_{c}", bufs=4)
        nc.tensor.matmul(
            fot_ps[:], mo_glob[:], h1ot[:, ts(c, 512)], start=True, stop=True
        )
        drain_copy(fot_sb[:, ts(c, 512)], fot_ps[:])
    if 6 in PHASES:
        nc.sync.dma_start(out=out_d[:], in_=fot_sb[:])


def build_program(reps=1, mode="rep", with_bias=False, loop=1):
    """Build the Bass program (shared by kernel() and test timing).

    loop > 1 wraps the body in an on-device For_i (timing amplification;
    only valid without collectives, i.e. mode="rep")."""
    key = (reps, mode, with_bias, loop, tuple(sorted(PHASES)))
    if key in _prog_cache:
        return _prog_cache[key]
    assert loop == 1 or mode == "rep", "device loop requires no collectives"

    _patch_tile_drain()
    import concourse.bass as bass
    import concourse.tile as tile
    import concourse.mybir as mybir
    from contextlib import ExitStack

    nloc = NLOC if mode == "shard" else N
    use_collectives = mode == "shard"

    f32 = mybir.dt.float32
    nc = bass.Bass(num_devices=N_CORES)
    tensors = {
        "xloc": nc.dram_tensor("xloc", [128, nloc], f32, kind="ExternalInput"),
        "w123t": nc.dram_tensor("w123t", [128, 3 * NHID], f32, kind="ExternalInput"),
        "b123": nc.dram_tensor("b123", [1, 3 * NHID], f32, kind="ExternalInput"),
        "wot_hi": nc.dram_tensor("wot_hi", [128, NCLASS], f32, kind="ExternalInput"),
        "wot_lo": nc.dram_tensor("wot_lo", [64, NCLASS], f32, kind="ExternalInput"),
        "bo": nc.dram_tensor("bo", [1, NCLASS], f32, kind="ExternalInput"),
        "outt": nc.dram_tensor("outt", [NCLASS, nloc], f32, kind="ExternalOutput"),
    }

    with tile.TileContext(nc) as tc:
        if loop > 1:
            with tc.For_i(0, loop, 1):
                for r in range(reps):
                    with ExitStack() as ctx:
                        _emit_body(
                            nc, tc, ctx, tensors, r, nloc, use_collectives, with_bias
                        )
        else:
            for r in range(reps):
                with ExitStack() as ctx:
                    _emit_body(
                        nc, tc, ctx, tensors, r, nloc, use_collectives, with_bias
                    )

    _split_multi_waits(nc)
    _prog_cache[key] = nc
    return nc


def make_in_maps(x, W1, b1, W2, b2, W3, b3, Wo, bo, mode="rep"):
    x_mem = np.asarray(x, dtype=np.float32).reshape(NFEAT, N)
    w123t = np.ascontiguousarray(
        np.concatenate(
            [np.asarray(W1).T, np.asarray(W2).T, np.asarray(W3).T], axis=1
        ),
        dtype=np.float32,
    )
    b123 = (
        np.concatenate([np.asarray(b1), np.asarray(b2), np.asarray(b3)])
        .reshape(1, 3 * NHID)
        .astype(np.float32)
    )
    wot = np.ascontiguousarray(np.asarray(Wo).T, dtype=np.float32)  # (192, 16)
    wot_hi = np.ascontiguousarray(wot[:128])
    wot_lo = np.ascontiguousarray(wot[128:])
    bo_r = np.asarray(bo).reshape(1, NCLASS).astype(np.float32)
    common = {
        "w123t": w123t,
        "b123": b123,
        "wot_hi": wot_hi,
        "wot_lo": wot_lo,
        "bo": bo_r,
    }
    in_maps = []
    for c in range(N_CORES):
        if mode == "shard":
            xc = np.ascontiguousarray(x_mem[:, c * NLOC : (c + 1) * NLOC])
        else:
            xc = x_mem
        in_maps.append({"xloc": xc, **common})
    return in_maps


def assemble_output(results, mode="rep"):
    if mode == "shard":
        slabs = [results[c]["outt"] for c in range(N_CORES)]
        full = np.concatenate(slabs, axis=1)  # (16, 4096)
    else:
        full = results[0]["outt"]
    return np.ascontiguousarray(full.reshape(1, NCLASS, 64, 64), dtype=np.float32)


def kernel(x, W1, b1, W2, b2, W3, b3, Wo, bo):
    from concourse.bass_utils import run_bass_kernel_spmd

    mode = "rep"
    with_bias = any(
        np.any(np.asarray(b)) for b in (b1, b2, b3, bo)
    )
    nc = build_program(reps=1, mode=mode, with_bias=with_bias)
    in_maps = make_in_maps(x, W1, b1, W2, b2, W3, b3, Wo, bo, mode=mode)
    res = run_bass_kernel_spmd(nc, in_maps, list(range(N_CORES)))
    return assemble_output(res.results, mode=mode)



# revision 2
# speedup vs baseline: 4.4172x; 4.4172x over previous
"""GAT kernel v3 for Trainium2, 8 NeuronCores, replicated (no collectives).

v2 -> v3: merged 1024-wide elu chunks, -1 folded into the elu min op
(removing 32 bias matmuls), engine-assignment knobs (DVE/ACT/Pool) for the
elementwise work, PSUM retagging to fit 8 banks with deeper z rotation.
"""

import sys

for _p in ("/opt/trn_rl_repo", "/root/.axon_site/_ro/trn_rl_repo"):
    if _p not in sys.path:
        sys.path.append(_p)

import numpy as np

N_CORES = 8
N = 4096
NT = 32
NFEAT = 128
NHID = 64
NCLASS = 16
D3 = 3 * NHID
DEFAULT_MODE = "rep"

# engine-assignment knobs (grid-searched via TimelineSim)
CFG = {
    "elu_max": "alt",            # "act" | "dve" | "alt"
    "elu_min": "dve",            # "dve" | "pool" | "alt"
    "drain1a": "flip",           # h_b drains: "act" | "flip"
    "h1s_eng": "dve",            # "dve" | "pool"
    "chain_grain": 24,           # 48 = half, 24 = quarter (in head-groups)
    "sq1a": "act",               # "act" | "split" (odd groups Pool from h_b)
    "norm1a": "dve",             # "dve" | "pool"
}

_prog_cache = {}


def _patch_tile_drain():
    import concourse.tile as tile
    from concourse.vector_clock import ScopedClock, VectorClock

    if getattr(tile.TileContext, "_drain_split_patched", False):
        return

    def _drain_and_barrier(self, tick_clock, wait_clock):
        nc = self.nc
        gvc = tick_clock.global_clock
        n = len(gvc)
        for proc in range(n):
            t = gvc[proc]
            if t > 0:
                sub = VectorClock([t if i == proc else 0 for i in range(n)])
                d = nc.sync.drain()
                wait_clock.add_sem_waits(d.ins, ScopedClock({None: sub}))
        nc.all_engine_barrier()
        assert self.sems is not None
        popped = nc._tile_sem_poison_stack.pop()
        assert popped is self._sem_poison
        nc.clear_and_free_semaphores(list(self.sems.allocated().values()))
        nc.all_engine_barrier()

    tile.TileContext._drain_and_barrier = _drain_and_barrier
    tile.TileContext._drain_split_patched = True


def _split_multi_waits(nc):
    import concourse.mybir as mybir

    n_new = 0
    for blk in nc.main_func.blocks:
        out = []
        changed = False
        for inst in blk.instructions:
            si = inst.sync_info
            waits = list(si.on_wait) if (si and si.on_wait) else []
            if len(waits) > 1:
                changed = True
                for w in waits[:-1]:
                    nop = mybir.InstNoOp(name=f"{inst.name}-xw{n_new}", ins=[], outs=[])
                    n_new += 1
                    nop.engine = inst.engine
                    nop.sync_info = mybir.SyncInfo(on_wait=[w], on_update=[])
                    out.append(nop)
                si.on_wait = [waits[-1]]
                inst.sync_info = si
            out.append(inst)
        if changed:
            blk.instructions = out


def _emit_body(nc, tc, ctx, tensors, r, with_bias):
    import concourse.bass as bass
    import concourse.mybir as mybir
    from concourse.bass import ts
    from concourse.masks import make_identity

    f32 = mybir.dt.float32
    bf16 = mybir.dt.float16
    AF = mybir.ActivationFunctionType

    const = ctx.enter_context(tc.tile_pool(name=f"const{r}", bufs=1))
    work = ctx.enter_context(tc.tile_pool(name=f"work{r}", bufs=1))
    small = ctx.enter_context(tc.tile_pool(name=f"small{r}", bufs=2))
    psum = ctx.enter_context(tc.tile_pool(name=f"psum{r}", bufs=1, space="PSUM"))

    def bcast_free(ap, inner):
        return bass.AP(tensor=ap.tensor, offset=ap.offset, ap=[*ap.ap, [0, inner]])

    flip = [0]

    def drain_copy(dst, src):
        if flip[0] & 1:
            nc.scalar.copy(dst, src)
        else:
            nc.vector.tensor_copy(dst, src)
        flip[0] += 1

    # =============== constants / inputs ===============
    w123t_sb = const.tile([128, D3], bf16, name=f"w123t{r}")
    nc.sync.dma_start(out=w123t_sb[:], in_=tensors["w123t"][:])
    xb_sb = const.tile([128, N], bf16, name=f"xb{r}")
    for c in range(8):
        nc.sync.dma_start(out=xb_sb[:, ts(c, 512)], in_=tensors["xb"][:, ts(c, 512)])
    wot_hi_sb = const.tile([128, NCLASS], bf16, name=f"wothi{r}")
    nc.sync.dma_start(out=wot_hi_sb[:], in_=tensors["wot_hi"][:])
    wot_lo_sb = const.tile([128, NCLASS], bf16, name=f"wotlo{r}")
    nc.sync.dma_start(out=wot_lo_sb[:], in_=tensors["wot_lo"][:])
    with_bias_h, with_bias_o = with_bias
    if with_bias_h:
        b123_sb = const.tile([1, D3], bf16, name=f"b123{r}")
        nc.sync.dma_start(out=b123_sb[:], in_=tensors["b123"][:])
    if with_bias_o:
        bo_sb = const.tile([1, NCLASS], bf16, name=f"bo{r}")
        nc.sync.dma_start(out=bo_sb[:], in_=tensors["bo16"][:])
    if with_bias_h or with_bias_o:
        ones1 = const.tile([1, 128], bf16, name=f"ones1{r}")
        nc.gpsimd.memset(ones1[:], 1.0)
    id128 = const.tile([128, 128], bf16, name=f"id128{r}")
    make_identity(nc, id128[:])

    # =============== persistent SBUF ===============
    h1a = const.tile([128, NT, D3], bf16, name=f"h1a{r}")
    h1t01 = const.tile([128, N], bf16, name=f"h1t01{r}")
    h1t2 = const.tile([NHID, N], bf16, name=f"h1t2{r}")
    hca_hi = const.tile([128, N], bf16, name=f"hcahi{r}")
    hca_lo = const.tile([NHID, N], bf16, name=f"hcalo{r}")
    hcm_hi = const.tile([128, N], bf16, name=f"hcmhi{r}")
    hcm_lo = const.tile([NHID, N], bf16, name=f"hcmlo{r}")
    sq = const.tile([128, NT, D3], bf16, name=f"sq{r}")
    s0 = const.tile([128, 96, 32], bf16, name=f"s0{r}")
    s1 = const.tile([128, 96, 16], bf16, name=f"s1{r}")
    h1s = const.tile([128, NT, D3], bf16, name=f"h1s{r}")
    gblk_b = const.tile([128, 128], bf16, name=f"gblk{r}")
    g2_b = const.tile([128, NHID], bf16, name=f"g2b{r}")
    mblk_b = const.tile([128, 128], bf16, name=f"mblk{r}")
    m2_b = const.tile([NHID, NHID], bf16, name=f"m2b{r}")
    q1 = const.tile([128, 96], f32, name=f"q1{r}")
    rn = const.tile([128, 96], bf16, name=f"rn{r}")
    q2 = const.tile([128, 96], f32, name=f"q2{r}")
    icn = const.tile([128, 96], bf16, name=f"icn{r}")

    def tree_reduce(sl, gw, q_t, rn_t, tag):
        """sl: slice index (units of gw tiles); gw: tiles per chunk.
        sq-style source given via tag's src view; reduces (p, gw*3, 64) ->
        q (f32) -> rn = sqrt(1/q) (bf16)."""
        pass  # inlined below

    def chain(src, lo, n3, q_t, rn_t, tag, srcname):
        """src: (128, *, 64)-viewable bf16; cols [lo*64.. ] over n3 groups.
        Writes q_t/rn_t[:, lo:lo+n3]."""
        nc.vector.tensor_add(
            s0[:, lo : lo + n3, :], src[:, :, 0:32], src[:, :, 32:64]
        )
        nc.vector.tensor_add(
            s1[:, lo : lo + n3, :],
            s0[:, lo : lo + n3, 0:16],
            s0[:, lo : lo + n3, 16:32],
        )
        nc.vector.reduce_sum(
            q_t[:, lo : lo + n3], s1[:, lo : lo + n3, :], axis=mybir.AxisListType.X
        )
        if CFG["rsqrt"] == "abs":
            nc.scalar.activation(
                rn_t[:, lo : lo + n3], q_t[:, lo : lo + n3],
                mybir.ActivationFunctionType.Abs_reciprocal_sqrt,
            )
        else:
            qi = small.tile(
                [128, n3], f32, tag=f"qi{tag}", name=f"qi{tag}{srcname}_{r}", bufs=4
            )
            nc.vector.reciprocal(qi[:], q_t[:, lo : lo + n3])
            nc.scalar.sqrt(rn_t[:, lo : lo + n3], qi[:])

    # ========== stage 1a + transposes + G, interleaved per half ==========
    h_b = const.tile([128, NT, D3], bf16, name=f"hb{r}")
    g01_t = psum.tile([128, 128], f32, tag="acc", name=f"gacc_{r}")
    g01_ps = g01_t[:]
    g2_t = psum.tile([NHID, NHID], f32, tag="accsm", name=f"g2acc_{r}")
    g2_ps = g2_t[:]
    for half in range(2):
        for g in range(4 * half, 4 * half + 4):
            ha = psum.tile([128, 4, 256], f32, tag="wide", name=f"ha{g}_{r}", bufs=2)
            for j in range(4):
                t = 4 * g + j
                nc.tensor.matmul(
                    ha[:, j, 0:D3], xb_sb[:, ts(t, 128)], w123t_sb[:],
                    start=True, stop=not with_bias_h,
                )
                if with_bias_h:
                    nc.tensor.matmul(
                        ha[:, j, 0:D3], ones1[:], b123_sb[:], start=False, stop=True
                    )
            if CFG["drain1a"] == "act":
                nc.scalar.copy(h_b[:, ts(g, 4), :], ha[:, :, 0:D3])
            else:
                drain_copy(h_b[:, ts(g, 4), :], ha[:, :, 0:D3])
            if CFG["sq1a"] == "split" and (g & 1):
                nc.gpsimd.tensor_mul(
                    sq[:, ts(g, 4), :], h_b[:, ts(g, 4), :], h_b[:, ts(g, 4), :]
                )
            else:
                nc.scalar.activation(sq[:, ts(g, 4), :], ha[:, :, 0:D3], AF.Square)
        gg = CFG["chain_grain"]
        for lo in range(half * 48, half * 48 + 48, gg):
            nt8 = gg // 3
            sqv = sq[:, lo // 3 : lo // 3 + nt8, :].rearrange(
                "p t (k d) -> p (t k) d", k=3
            )
            chain(sqv, lo, gg, q1, rn, "a", f"c{lo}")
            eng_n1 = nc.gpsimd if CFG["norm1a"] == "pool" else nc.vector
            eng_n1.tensor_mul(
                h1a[:, lo // 3 : lo // 3 + nt8, :].rearrange(
                    "p t (k d) -> p (t k) d", k=3
                ),
                h_b[:, lo // 3 : lo // 3 + nt8, :].rearrange(
                    "p t (k d) -> p (t k) d", k=3
                ),
                bcast_free(rn[:, lo : lo + gg], NHID),
            )
        for gp in range(2 * half, 2 * half + 2):
            tr = psum.tile([128, 8, 128], bf16, tag="z", name=f"tr{gp}a_{r}", bufs=2)
            for j in range(4):
                t = 8 * gp + j
                nc.tensor.transpose(tr[:, j, :], h1a[:, t, 0:128], id128[:])
                nc.tensor.transpose(tr[0:NHID, 4 + j, :], h1a[:, t, 128:D3], id128[:])
            tr2 = psum.tile([128, 8, 128], bf16, tag="z", name=f"tr{gp}b_{r}", bufs=2)
            for j in range(4):
                t = 8 * gp + 4 + j
                nc.tensor.transpose(tr2[:, j, :], h1a[:, t, 0:128], id128[:])
                nc.tensor.transpose(tr2[0:NHID, 4 + j, :], h1a[:, t, 128:D3], id128[:])
            drain_copy(
                h1t01[:, ts(2 * gp, 512)].rearrange("p (a b) -> p a b", a=4),
                tr[:, 0:4, :],
            )
            drain_copy(
                h1t01[:, ts(2 * gp + 1, 512)].rearrange("p (a b) -> p a b", a=4),
                tr2[:, 0:4, :],
            )
            drain_copy(
                h1t2[:, ts(2 * gp, 512)].rearrange("p (a b) -> p a b", a=4),
                tr[0:NHID, 4:8, :],
            )
            drain_copy(
                h1t2[:, ts(2 * gp + 1, 512)].rearrange("p (a b) -> p a b", a=4),
                tr2[0:NHID, 4:8, :],
            )
            for j in range(8):
                t = 8 * gp + j
                nc.tensor.matmul(
                    g01_ps, h1a[:, t, 0:128], h1a[:, t, 0:128],
                    start=(t == 0), stop=(t == NT - 1),
                )
                nc.tensor.matmul(
                    g2_ps, h1a[:, t, 128:D3], h1a[:, t, 128:D3],
                    start=(t == 0), stop=(t == NT - 1),
                )
    nc.gpsimd.memset(gblk_b[:], 0.0)
    nc.vector.tensor_copy(gblk_b[0:NHID, 0:NHID], g01_ps[0:NHID, 0:NHID])
    nc.scalar.copy(gblk_b[NHID:128, NHID:128], g01_ps[NHID:128, NHID:128])
    nc.vector.tensor_copy(g2_b[0:NHID, :], g2_ps)
    nc.scalar.copy(g2_b[NHID:128, :], g2_ps)

    # =============== stage 1b: ta, colnorm, M ===============
    for g in range(8):
        ta = psum.tile([128, 4, 256], f32, tag="wide", name=f"ta{g}_{r}", bufs=2)
        for j in range(4):
            t = 4 * g + j
            nc.tensor.matmul(
                ta[:, j, 0:128], h1t01[:, ts(t, 128)], gblk_b[:],
                start=True, stop=True,
            )
            nc.tensor.matmul(
                ta[:, j, 128:D3], h1t2[:, ts(t, 128)], g2_b[:],
                start=True, stop=True,
            )
        nc.vector.tensor_mul(
            sq[:, ts(g, 4), :], ta[:, :, 0:D3], h1a[:, ts(g, 4), :]
        )
    m01_t = psum.tile([128, 128], f32, tag="acc", name=f"macc_{r}")
    m01_ps = m01_t[:]
    m2_t = psum.tile([NHID, NHID], f32, tag="accsm", name=f"m2acc_{r}")
    m2_ps = m2_t[:]
    gg = CFG["chain_grain"]
    eng_h1s = nc.gpsimd if CFG["h1s_eng"] == "pool" else nc.vector
    for lo in range(0, 96, gg):
        nt8 = gg // 3
        t0 = lo // 3
        sqv = sq[:, t0 : t0 + nt8, :].rearrange("p t (k d) -> p (t k) d", k=3)
        chain(sqv, lo, gg, q2, icn, "b", f"c{lo}")
        eng_h1s.tensor_mul(
            h1s[:, t0 : t0 + nt8, :].rearrange("p t (k d) -> p (t k) d", k=3),
            h1a[:, t0 : t0 + nt8, :].rearrange("p t (k d) -> p (t k) d", k=3),
            bcast_free(icn[:, lo : lo + gg], NHID),
        )
        for t in range(t0, t0 + nt8):
            nc.tensor.matmul(
                m01_ps, h1a[:, t, 0:128], h1s[:, t, 0:128],
                start=(t == 0), stop=(t == NT - 1),
            )
            nc.tensor.matmul(
                m2_ps, h1a[:, t, 128:D3], h1s[:, t, 128:D3],
                start=(t == 0), stop=(t == NT - 1),
            )
    nc.gpsimd.memset(mblk_b[:], 0.0)
    nc.vector.tensor_copy(mblk_b[0:NHID, 0:NHID], m01_ps[0:NHID, 0:NHID])
    nc.scalar.copy(mblk_b[NHID:128, NHID:128], m01_ps[NHID:128, NHID:128])
    nc.vector.tensor_copy(m2_b[:], m2_ps)

    # =============== z = h1 @ M (transposed), elu -> hcT ===============
    # elu(z) = max(z,0) + (min(exp(z),1) - 1); min+add fused in one DVE op.
    def elu_chunk(z_ps, parts, dst_a, dst_m, cidx):
        e1 = work.tile(
            [parts, 2, 512], bf16, tag=f"e1{parts}", name=f"e1_{cidx}_{r}", bufs=3
        )
        nc.scalar.activation(e1[:], z_ps[:], AF.Exp)
        if CFG["elu_max"] == "act" or (CFG["elu_max"] == "alt" and cidx & 1):
            nc.scalar.activation(dst_a, z_ps[:], AF.Relu)
        else:
            nc.vector.tensor_scalar_max(dst_a, z_ps[:], 0.0)
        mn = CFG["elu_min"]
        eng_min = nc.gpsimd if (mn == "pool" or (mn == "alt" and cidx & 1)) else nc.vector
        eng_min.tensor_scalar(
            out=dst_m, in0=e1[:], scalar1=1.0, scalar2=-1.0,
            op0=mybir.AluOpType.min, op1=mybir.AluOpType.add,
        )
        if CFG["hc_add"] != "none":
            eng_add = nc.gpsimd if CFG["hc_add"] == "pool" else nc.vector
            eng_add.tensor_tensor(
                out=dst_a, in0=dst_a, in1=dst_m, op=mybir.AluOpType.add
            )

    ho_ps = psum.tile([128, NT, NCLASS], f32, tag="acc", name=f"ho_{r}")
    for c in range(4):
        zhi = psum.tile([128, 2, 512], f32, tag="wide", name=f"zhi{c}_{r}", bufs=2)
        for i in range(2):
            nc.tensor.matmul(
                zhi[:, i, :], mblk_b[:], h1t01[:, ts(2 * c + i, 512)],
                start=True, stop=True,
            )
        elu_chunk(
            zhi, 128,
            hca_hi[:, ts(c, 1024)].rearrange("p (a b) -> p a b", a=2),
            hcm_hi[:, ts(c, 1024)].rearrange("p (a b) -> p a b", a=2),
            2 * c,
        )
        zlo = psum.tile([NHID, 2, 512], f32, tag="wide", name=f"zlo{c}_{r}", bufs=2)
        for i in range(2):
            nc.tensor.matmul(
                zlo[:, i, :], m2_b[:], h1t2[:, ts(2 * c + i, 512)],
                start=True, stop=True,
            )
        elu_chunk(
            zlo, NHID,
            hca_lo[:, ts(c, 1024)].rearrange("p (a b) -> p a b", a=2),
            hcm_lo[:, ts(c, 1024)].rearrange("p (a b) -> p a b", a=2),
            2 * c + 1,
        )
        for t in range(8 * c, 8 * c + 8):
            nc.tensor.matmul(
                ho_ps[:, t, :], hca_hi[:, ts(t, 128)], wot_hi_sb[:],
                start=True, stop=False,
            )
            nc.tensor.matmul(
                ho_ps[:, t, :], hcm_hi[:, ts(t, 128)], wot_hi_sb[:],
                start=False, stop=False,
            )
            nc.tensor.matmul(
                ho_ps[:, t, :], hca_lo[:, ts(t, 128)], wot_lo_sb[:],
                start=False, stop=False,
            )
            nc.tensor.matmul(
                ho_ps[:, t, :], hcm_lo[:, ts(t, 128)], wot_lo_sb[:],
                start=False, stop=not with_bias_o,
            )
            if with_bias_o:
                nc.tensor.matmul(
                    ho_ps[:, t, :], ones1[:], bo_sb[:], start=False, stop=True
                )

    # (ho accumulated inside the elu loop above)
    sqo = const.tile([128, NT, NCLASS], bf16, name=f"sqo{r}")
    qo = const.tile([128, NT], f32, name=f"qo{r}")
    rno = const.tile([128, NT], bf16, name=f"rno{r}")
    h1oa = const.tile([128, NT, NCLASS], bf16, name=f"h1oa{r}")
    for half in range(2):
        nc.scalar.activation(
            sqo[:, ts(half, 16), :], ho_ps[:, ts(half, 16), :], AF.Square
        )
        nc.vector.reduce_sum(
            qo[:, ts(half, 16)], sqo[:, ts(half, 16), :], axis=mybir.AxisListType.X
        )
        if CFG["rsqrt"] == "abs":
            nc.scalar.activation(
                rno[:, ts(half, 16)], qo[:, ts(half, 16)],
                mybir.ActivationFunctionType.Abs_reciprocal_sqrt,
            )
        else:
            qoi = small.tile([128, 16], f32, tag="qoi", name=f"qoi{half}_{r}")
            nc.vector.reciprocal(qoi[:], qo[:, ts(half, 16)])
            nc.scalar.sqrt(rno[:, ts(half, 16)], qoi[:])
        nc.vector.tensor_mul(
            h1oa[:, ts(half, 16), :],
            ho_ps[:, ts(half, 16), :],
            bcast_free(rno[:, ts(half, 16)], NCLASS),
        )
    go_t = psum.tile([NCLASS, NCLASS], f32, tag="accsm", name=f"goacc_{r}")
    go_ps = go_t[:]
    h1ot = const.tile([NCLASS, N], bf16, name=f"h1ot{r}")
    for gp in range(4):
        tro = psum.tile([NCLASS, 8, 128], bf16, tag="z", name=f"tro{gp}_{r}", bufs=2)
        for j in range(8):
            t = 8 * gp + j
            nc.tensor.transpose(tro[:, j, :], h1oa[:, t, :], id128[:])
            nc.tensor.matmul(
                go_ps, h1oa[:, t, :], h1oa[:, t, :],
                start=(t == 0), stop=(t == NT - 1),
            )
        drain_copy(
            h1ot[:, ts(gp, 1024)].rearrange("p (a b) -> p a b", a=8), tro[:]
        )
    go_b = const.tile([NCLASS, NCLASS], bf16, name=f"gob{r}")
    nc.vector.tensor_copy(go_b[:], go_ps)

    to_ps = psum.tile([128, NT, NCLASS], f32, tag="acc", name=f"to_{r}")
    for t in range(NT):
        nc.tensor.matmul(
            to_ps[:, t, :], h1ot[:, ts(t, 128)], go_b[:], start=True, stop=True
        )
    scro = const.tile([128, NT, NCLASS], bf16, name=f"scro{r}")
    nc.vector.tensor_mul(scro[:], to_ps[:], h1oa[:])
    qo2 = const.tile([128, NT], f32, name=f"qo2{r}")
    nc.vector.reduce_sum(qo2[:], scro[:], axis=mybir.AxisListType.X)
    icno = const.tile([128, NT], bf16, name=f"icno{r}")
    if CFG["rsqrt"] == "abs":
        nc.scalar.activation(
            icno[:], qo2[:], mybir.ActivationFunctionType.Abs_reciprocal_sqrt
        )
    else:
        qo2i = small.tile([128, NT], f32, tag="qo2i", name=f"qo2i{r}")
        nc.vector.reciprocal(qo2i[:], qo2[:])
        nc.scalar.sqrt(icno[:], qo2i[:])
    h1so = const.tile([128, NT, NCLASS], bf16, name=f"h1so{r}")
    nc.vector.tensor_mul(h1so[:], h1oa[:], bcast_free(icno[:], NCLASS))
    mo_t = psum.tile([NCLASS, NCLASS], f32, tag="accsm", name=f"moacc_{r}")
    mo_ps = mo_t[:]
    for t in range(NT):
        nc.tensor.matmul(
            mo_ps, h1oa[:, t, :], h1so[:, t, :],
            start=(t == 0), stop=(t == NT - 1),
        )
    mo_b = const.tile([NCLASS, NCLASS], bf16, name=f"mob{r}")
    nc.vector.tensor_copy(mo_b[:], mo_ps)

    fo_ps = psum.tile([128, NT, NCLASS], f32, tag="acc", name=f"fo_{r}")
    for t in range(NT):
        nc.tensor.matmul(
            fo_ps[:, t, :], h1ot[:, ts(t, 128)], mo_b[:], start=True, stop=True
        )
    fot_sb = const.tile([128, NT * NCLASS], f32, name=f"fot{r}")
    drain_copy(fot_sb[:, 0:256].rearrange("p (a b) -> p a b", a=16), fo_ps[:, 0:16, :])
    nc.sync.dma_start(out=tensors["outt"][:, 0:256], in_=fot_sb[:, 0:256])
    drain_copy(fot_sb[:, 256:512].rearrange("p (a b) -> p a b", a=16), fo_ps[:, 16:32, :])
    nc.sync.dma_start(out=tensors["outt"][:, 256:512], in_=fot_sb[:, 256:512])


def build_program(reps=1, mode="rep", with_bias=(False, False), loop=1):
    key = (reps, mode, with_bias, loop, tuple(sorted(CFG.items())))
    if key in _prog_cache:
        return _prog_cache[key]

    _patch_tile_drain()
    import concourse.bass as bass
    import concourse.tile as tile
    import concourse.mybir as mybir
    from contextlib import ExitStack

    f32 = mybir.dt.float32
    bf16 = mybir.dt.float16
    nc = bass.Bass(num_devices=N_CORES)
    tensors = {
        "xb": nc.dram_tensor("xb", [128, N], bf16, kind="ExternalInput"),
        "w123t": nc.dram_tensor("w123t", [128, D3], bf16, kind="ExternalInput"),
        "wot_hi": nc.dram_tensor("wot_hi", [128, NCLASS], bf16, kind="ExternalInput"),
        "wot_lo": nc.dram_tensor("wot_lo", [128, NCLASS], bf16, kind="ExternalInput"),
        "b123": nc.dram_tensor("b123", [1, D3], bf16, kind="ExternalInput"),
        "bo16": nc.dram_tensor("bo16", [1, NCLASS], bf16, kind="ExternalInput"),
        "outt": nc.dram_tensor("outt", [128, NT * NCLASS], f32, kind="ExternalOutput"),
    }

    with tile.TileContext(nc) as tc:
        if loop > 1:
            with tc.For_i(0, loop, 1):
                for r in range(reps):
                    with ExitStack() as ctx:
                        _emit_body(nc, tc, ctx, tensors, r, with_bias)
        else:
            for r in range(reps):
                with ExitStack() as ctx:
                    _emit_body(nc, tc, ctx, tensors, r, with_bias)

    _split_multi_waits(nc)
    _prog_cache[key] = nc
    return nc


def make_in_maps(x, W1, b1, W2, b2, W3, b3, Wo, bo, mode="rep"):
    import ml_dtypes

    bf = np.float16
    x_mem = np.asarray(x, dtype=np.float32).reshape(NFEAT, N)
    w123t = np.ascontiguousarray(
        np.concatenate(
            [np.asarray(W1).T, np.asarray(W2).T, np.asarray(W3).T], axis=1
        ),
        dtype=np.float32,
    )
    wot = np.ascontiguousarray(np.asarray(Wo).T, dtype=np.float32)
    b123 = (
        np.concatenate([np.asarray(b1), np.asarray(b2), np.asarray(b3)])
        .reshape(1, D3)
        .astype(bf)
    )
    common = {
        "xb": x_mem.astype(bf),
        "w123t": w123t.astype(bf),
        "wot_hi": np.ascontiguousarray(wot[:128]).astype(bf),
        "wot_lo": np.ascontiguousarray(np.concatenate([wot[128:], wot[128:]], axis=0)).astype(bf),
        "b123": b123,
        "bo16": np.asarray(bo).reshape(1, NCLASS).astype(bf),
    }
    return [dict(common) for _ in range(N_CORES)]


def assemble_output(results, mode="rep"):
    fot = np.asarray(results[0]["outt"], dtype=np.float32)
    h = fot.reshape(128, NT, NCLASS).transpose(1, 0, 2).reshape(N, NCLASS)
    return np.ascontiguousarray(h.T.reshape(1, NCLASS, 64, 64), dtype=np.float32)


def kernel(x, W1, b1, W2, b2, W3, b3, Wo, bo):
    from concourse.bass_utils import run_bass_kernel_spmd

    with_bias = (
        any(np.any(np.asarray(b)) for b in (b1, b2, b3)),
        bool(np.any(np.asarray(bo))),
    )
    nc = build_program(reps=1, mode="rep", with_bias=with_bias)
    in_maps = make_in_maps(x, W1, b1, W2, b2, W3, b3, Wo, bo)
    res = run_bass_kernel_spmd(nc, in_maps, list(range(N_CORES)))
    return assemble_output(res.results)


# revision 4
# speedup vs baseline: 12.3740x; 2.8013x over previous
"""GAT kernel v3 for Trainium2, 8 NeuronCores, replicated (no collectives).

v2 -> v3: merged 1024-wide elu chunks, -1 folded into the elu min op
(removing 32 bias matmuls), engine-assignment knobs (DVE/ACT/Pool) for the
elementwise work, PSUM retagging to fit 8 banks with deeper z rotation.
"""

import sys

for _p in ("/opt/trn_rl_repo", "/root/.axon_site/_ro/trn_rl_repo"):
    if _p not in sys.path:
        sys.path.append(_p)

import numpy as np

N_CORES = 8
N = 4096
NT = 32
NFEAT = 128
NHID = 64
NCLASS = 16
D3 = 3 * NHID
DEFAULT_MODE = "rep"

# engine-assignment knobs (grid-searched via TimelineSim)
CFG = {
    "elu_max": "alt",            # "act" | "dve" | "alt"
    "elu_min": "dve",            # "dve" | "pool" | "alt"
    "drain1a": "flip",           # h_b drains: "act" | "flip"
    "h1s_eng": "dve",            # "dve" | "pool"
    "chain_grain": 24,           # 48 = half, 24 = quarter (in head-groups)
    "sq1a": "act",               # "act" | "split" (odd groups Pool from h_b)
    "norm1a": "dve",             # "dve" | "pool"
}

_prog_cache = {}


def _patch_tile_drain():
    import concourse.tile as tile
    from concourse.vector_clock import ScopedClock, VectorClock

    if getattr(tile.TileContext, "_drain_split_patched", False):
        return

    def _drain_and_barrier(self, tick_clock, wait_clock):
        nc = self.nc
        gvc = tick_clock.global_clock
        n = len(gvc)
        for proc in range(n):
            t = gvc[proc]
            if t > 0:
                sub = VectorClock([t if i == proc else 0 for i in range(n)])
                d = nc.sync.drain()
                wait_clock.add_sem_waits(d.ins, ScopedClock({None: sub}))
        nc.all_engine_barrier()
        assert self.sems is not None
        popped = nc._tile_sem_poison_stack.pop()
        assert popped is self._sem_poison
        nc.clear_and_free_semaphores(list(self.sems.allocated().values()))
        nc.all_engine_barrier()

    tile.TileContext._drain_and_barrier = _drain_and_barrier
    tile.TileContext._drain_split_patched = True


def _split_multi_waits(nc):
    import concourse.mybir as mybir

    n_new = 0
    for blk in nc.main_func.blocks:
        out = []
        changed = False
        for inst in blk.instructions:
            si = inst.sync_info
            waits = list(si.on_wait) if (si and si.on_wait) else []
            if len(waits) > 1:
                changed = True
                for w in waits[:-1]:
                    nop = mybir.InstNoOp(name=f"{inst.name}-xw{n_new}", ins=[], outs=[])
                    n_new += 1
                    nop.engine = inst.engine
                    nop.sync_info = mybir.SyncInfo(on_wait=[w], on_update=[])
                    out.append(nop)
                si.on_wait = [waits[-1]]
                inst.sync_info = si
            out.append(inst)
        if changed:
            blk.instructions = out


def _emit_body(nc, tc, ctx, tensors, r, with_bias):
    import concourse.bass as bass
    import concourse.mybir as mybir
    from concourse.bass import ts
    from concourse.masks import make_identity

    f32 = mybir.dt.float32
    bf16 = mybir.dt.float16
    AF = mybir.ActivationFunctionType

    const = ctx.enter_context(tc.tile_pool(name=f"const{r}", bufs=1))
    work = ctx.enter_context(tc.tile_pool(name=f"work{r}", bufs=1))
    small = ctx.enter_context(tc.tile_pool(name=f"small{r}", bufs=2))
    psum = ctx.enter_context(tc.tile_pool(name=f"psum{r}", bufs=1, space="PSUM"))

    def bcast_free(ap, inner):
        return bass.AP(tensor=ap.tensor, offset=ap.offset, ap=[*ap.ap, [0, inner]])

    flip = [0]

    def drain_copy(dst, src):
        if flip[0] & 1:
            nc.scalar.copy(dst, src)
        else:
            nc.vector.tensor_copy(dst, src)
        flip[0] += 1

    # =============== constants / inputs ===============
    w123t_sb = const.tile([128, D3], bf16, name=f"w123t{r}")
    nc.sync.dma_start(out=w123t_sb[:], in_=tensors["w123t"][:])
    xb_sb = const.tile([128, N], bf16, name=f"xb{r}")
    for c in range(8):
        nc.sync.dma_start(out=xb_sb[:, ts(c, 512)], in_=tensors["xb"][:, ts(c, 512)])
    wot_hi_sb = const.tile([128, NCLASS], bf16, name=f"wothi{r}")
    nc.sync.dma_start(out=wot_hi_sb[:], in_=tensors["wot_hi"][:])
    wot_lo_sb = const.tile([128, NCLASS], bf16, name=f"wotlo{r}")
    nc.sync.dma_start(out=wot_lo_sb[:], in_=tensors["wot_lo"][:])
    with_bias_h, with_bias_o = with_bias
    if with_bias_h:
        b123_sb = const.tile([1, D3], bf16, name=f"b123{r}")
        nc.sync.dma_start(out=b123_sb[:], in_=tensors["b123"][:])
    if with_bias_o:
        bo_sb = const.tile([1, NCLASS], bf16, name=f"bo{r}")
        nc.sync.dma_start(out=bo_sb[:], in_=tensors["bo16"][:])
    if with_bias_h or with_bias_o:
        ones1 = const.tile([1, 128], bf16, name=f"ones1{r}")
        nc.gpsimd.memset(ones1[:], 1.0)
    id128 = const.tile([128, 128], bf16, name=f"id128{r}")
    make_identity(nc, id128[:])
    _EARLY_MEMSET = True

    # =============== persistent SBUF ===============
    h1a = const.tile([128, NT, D3], bf16, name=f"h1a{r}")
    h1t01 = const.tile([128, N], bf16, name=f"h1t01{r}")
    h1t2 = const.tile([NHID, N], bf16, name=f"h1t2{r}")
    hca_hi = const.tile([128, N], bf16, name=f"hcahi{r}")
    hca_lo = const.tile([NHID, N], bf16, name=f"hcalo{r}")
    hcm_hi = const.tile([128, N], bf16, name=f"hcmhi{r}")
    hcm_lo = const.tile([NHID, N], bf16, name=f"hcmlo{r}")
    sq = const.tile([128, NT, D3], bf16, name=f"sq{r}")
    s0 = const.tile([128, 96, 32], bf16, name=f"s0{r}")
    s1 = const.tile([128, 96, 16], bf16, name=f"s1{r}")
    h1s = const.tile([128, NT, D3], bf16, name=f"h1s{r}")
    gblk_b = const.tile([128, 128], bf16, name=f"gblk{r}")
    g2_b = const.tile([128, NHID], bf16, name=f"g2b{r}")
    mblk_b = const.tile([128, 128], bf16, name=f"mblk{r}")
    m2bd = const.tile([128, 128], bf16, name=f"m2bd{r}")
    nc.gpsimd.memset(gblk_b[:], 0.0)
    nc.gpsimd.memset(mblk_b[:], 0.0)
    nc.gpsimd.memset(m2bd[:], 0.0)
    q1 = const.tile([128, 96], f32, name=f"q1{r}")
    rn = const.tile([128, 96], bf16, name=f"rn{r}")
    q2 = const.tile([128, 96], f32, name=f"q2{r}")
    icn = const.tile([128, 96], bf16, name=f"icn{r}")

    def tree_reduce(sl, gw, q_t, rn_t, tag):
        """sl: slice index (units of gw tiles); gw: tiles per chunk.
        sq-style source given via tag's src view; reduces (p, gw*3, 64) ->
        q (f32) -> rn = sqrt(1/q) (bf16)."""
        pass  # inlined below

    def chain(src, lo, n3, q_t, rn_t, tag, srcname):
        """src: (128, *, 64)-viewable bf16; cols [lo*64.. ] over n3 groups.
        Writes q_t/rn_t[:, lo:lo+n3]."""
        nc.vector.tensor_add(
            s0[:, lo : lo + n3, :], src[:, :, 0:32], src[:, :, 32:64]
        )
        nc.vector.tensor_add(
            s1[:, lo : lo + n3, :],
            s0[:, lo : lo + n3, 0:16],
            s0[:, lo : lo + n3, 16:32],
        )
        nc.vector.reduce_sum(
            q_t[:, lo : lo + n3], s1[:, lo : lo + n3, :], axis=mybir.AxisListType.X
        )
        if CFG["rsqrt"] == "abs":
            nc.scalar.activation(
                rn_t[:, lo : lo + n3], q_t[:, lo : lo + n3],
                mybir.ActivationFunctionType.Abs_reciprocal_sqrt,
            )
        else:
            qi = small.tile(
                [128, n3], f32, tag=f"qi{tag}", name=f"qi{tag}{srcname}_{r}", bufs=4
            )
            eng_rc = nc.gpsimd if CFG.get("recip_eng") == "pool" else nc.vector
            eng_rc.reciprocal(qi[:], q_t[:, lo : lo + n3])
            nc.scalar.sqrt(rn_t[:, lo : lo + n3], qi[:])

    # ========== stage 1a + transposes + G, interleaved per half ==========
    h_b = const.tile([128, NT, D3], bf16, name=f"hb{r}")
    g01_t = psum.tile([128, 128], f32, tag="acc", name=f"gacc_{r}")
    g01_ps = g01_t[:]
    g2_t = psum.tile([NHID, NHID], f32, tag="accsm", name=f"g2acc_{r}")
    g2_ps = g2_t[:]
    for half in range(2):
        for g in range(4 * half, 4 * half + 4):
            ha = psum.tile([128, 4, 256], f32, tag="wide", name=f"ha{g}_{r}", bufs=2)
            for j in range(4):
                t = 4 * g + j
                nc.tensor.matmul(
                    ha[:, j, 0:D3], xb_sb[:, ts(t, 128)], w123t_sb[:],
                    start=True, stop=not with_bias_h,
                )
                if with_bias_h:
                    nc.tensor.matmul(
                        ha[:, j, 0:D3], ones1[:], b123_sb[:], start=False, stop=True
                    )
            if CFG["drain1a"] == "act":
                nc.scalar.copy(h_b[:, ts(g, 4), :], ha[:, :, 0:D3])
            else:
                drain_copy(h_b[:, ts(g, 4), :], ha[:, :, 0:D3])
            if CFG["sq1a"] == "split" and (g & 1):
                nc.gpsimd.tensor_mul(
                    sq[:, ts(g, 4), :], h_b[:, ts(g, 4), :], h_b[:, ts(g, 4), :]
                )
            else:
                nc.scalar.activation(sq[:, ts(g, 4), :], ha[:, :, 0:D3], AF.Square)
        gg = CFG["chain_grain"]
        for lo in range(half * 48, half * 48 + 48, gg):
            nt8 = gg // 3
            sqv = sq[:, lo // 3 : lo // 3 + nt8, :].rearrange(
                "p t (k d) -> p (t k) d", k=3
            )
            chain(sqv, lo, gg, q1, rn, "a", f"c{lo}")
            eng_n1 = nc.gpsimd if CFG["norm1a"] == "pool" else nc.vector
            eng_n1.tensor_mul(
                h1a[:, lo // 3 : lo // 3 + nt8, :].rearrange(
                    "p t (k d) -> p (t k) d", k=3
                ),
                h_b[:, lo // 3 : lo // 3 + nt8, :].rearrange(
                    "p t (k d) -> p (t k) d", k=3
                ),
                bcast_free(rn[:, lo : lo + gg], NHID),
            )
        for gp in range(2 * half, 2 * half + 2):
            tr = psum.tile([128, 8, 128], bf16, tag="z", name=f"tr{gp}a_{r}", bufs=2)
            for j in range(4):
                t = 8 * gp + j
                nc.tensor.transpose(tr[:, j, :], h1a[:, t, 0:128], id128[:])
                nc.tensor.transpose(tr[0:NHID, 4 + j, :], h1a[:, t, 128:D3], id128[:])
            tr2 = psum.tile([128, 8, 128], bf16, tag="z", name=f"tr{gp}b_{r}", bufs=2)
            for j in range(4):
                t = 8 * gp + 4 + j
                nc.tensor.transpose(tr2[:, j, :], h1a[:, t, 0:128], id128[:])
                nc.tensor.transpose(tr2[0:NHID, 4 + j, :], h1a[:, t, 128:D3], id128[:])
            drain_copy(
                h1t01[:, ts(2 * gp, 512)].rearrange("p (a b) -> p a b", a=4),
                tr[:, 0:4, :],
            )
            drain_copy(
                h1t01[:, ts(2 * gp + 1, 512)].rearrange("p (a b) -> p a b", a=4),
                tr2[:, 0:4, :],
            )
            drain_copy(
                h1t2[:, ts(2 * gp, 512)].rearrange("p (a b) -> p a b", a=4),
                tr[0:NHID, 4:8, :],
            )
            drain_copy(
                h1t2[:, ts(2 * gp + 1, 512)].rearrange("p (a b) -> p a b", a=4),
                tr2[0:NHID, 4:8, :],
            )
            for j in range(8):
                t = 8 * gp + j
                nc.tensor.matmul(
                    g01_ps, h1a[:, t, 0:128], h1a[:, t, 0:128],
                    start=(t == 0), stop=(t == NT - 1),
                )
                nc.tensor.matmul(
                    g2_ps, h1a[:, t, 128:D3], h1a[:, t, 128:D3],
                    start=(t == 0), stop=(t == NT - 1),
                )
    nc.vector.tensor_copy(gblk_b[0:NHID, 0:NHID], g01_ps[0:NHID, 0:NHID])
    nc.scalar.copy(gblk_b[NHID:128, NHID:128], g01_ps[NHID:128, NHID:128])
    nc.vector.tensor_copy(g2_b[0:NHID, :], g2_ps)
    nc.scalar.copy(g2_b[NHID:128, :], g2_ps)

    # =============== stage 1b: ta, colnorm, M ===============
    for g in range(8):
        ta = psum.tile([128, 4, 256], f32, tag="wide", name=f"ta{g}_{r}", bufs=2)
        for j in range(4):
            t = 4 * g + j
            nc.tensor.matmul(
                ta[:, j, 0:128], h1t01[:, ts(t, 128)], gblk_b[:],
                start=True, stop=True,
            )
            _c = t >> 2
            nc.tensor.matmul(
                ta[:, j, 128:D3],
                h1t2[
                    64 * (_c & 1) : 64 * (_c & 1) + 64,
                    (_c >> 1) * 512 + (t & 3) * 128 : (_c >> 1) * 512 + (t & 3) * 128 + 128,
                ],
                g2_b[64 * (_c & 1) : 64 * (_c & 1) + 64, :],
                start=True, stop=True,
            )
        if CFG["scr2_drain"] == "act":
            tad = work.tile(
                [128, 4, D3], bf16, tag="tad", name=f"tad{g}_{r}", bufs=3
            )
            nc.scalar.copy(tad[:], ta[:, :, 0:D3])
            nc.vector.tensor_mul(sq[:, ts(g, 4), :], tad[:], h1a[:, ts(g, 4), :])
        else:
            nc.vector.tensor_mul(
                sq[:, ts(g, 4), :], ta[:, :, 0:D3], h1a[:, ts(g, 4), :]
            )
    m01_t = psum.tile([128, 128], f32, tag="acc", name=f"macc_{r}")
    m01_ps = m01_t[:]
    m2_t = psum.tile([NHID, NHID], f32, tag="accsm", name=f"m2acc_{r}")
    m2_ps = m2_t[:]
    gg = CFG["chain_grain"]
    eng_h1s = nc.gpsimd if CFG["h1s_eng"] == "pool" else nc.vector
    for lo in range(0, 96, gg):
        nt8 = gg // 3
        t0 = lo // 3
        sqv = sq[:, t0 : t0 + nt8, :].rearrange("p t (k d) -> p (t k) d", k=3)
        chain(sqv, lo, gg, q2, icn, "b", f"c{lo}")
        eng_h1s.tensor_mul(
            h1s[:, t0 : t0 + nt8, :].rearrange("p t (k d) -> p (t k) d", k=3),
            h1a[:, t0 : t0 + nt8, :].rearrange("p t (k d) -> p (t k) d", k=3),
            bcast_free(icn[:, lo : lo + gg], NHID),
        )
        for t in range(t0, t0 + nt8):
            nc.tensor.matmul(
                m01_ps, h1a[:, t, 0:128], h1s[:, t, 0:128],
                start=(t == 0), stop=(t == NT - 1),
            )
            nc.tensor.matmul(
                m2_ps, h1a[:, t, 128:D3], h1s[:, t, 128:D3],
                start=(t == 0), stop=(t == NT - 1),
            )
    nc.vector.tensor_copy(mblk_b[0:NHID, 0:NHID], m01_ps[0:NHID, 0:NHID])
    nc.scalar.copy(mblk_b[NHID:128, NHID:128], m01_ps[NHID:128, NHID:128])

    # =============== z = h1 @ M (transposed), elu -> hcT ===============
    # elu(z) = max(z,0) + (min(exp(z),1) - 1); min+add fused in one DVE op.
    def elu_chunk_flat(z_ps, dst_a, dst_m, cidx):
        e1 = work.tile(
            [128, 512], bf16, tag="e1f", name=f"e1f_{cidx}_{r}", bufs=3
        )
        nc.scalar.activation(e1[:], z_ps[:], AF.Exp)
        if CFG["elu_max"] == "act" or (CFG["elu_max"] == "alt" and cidx & 1):
            nc.scalar.activation(dst_a, z_ps[:], AF.Relu)
        else:
            nc.vector.tensor_scalar_max(dst_a, z_ps[:], 0.0)
        mn = CFG["elu_min"]
        eng_min = nc.gpsimd if (mn == "pool" or (mn == "alt" and cidx & 1)) else nc.vector
        eng_min.tensor_scalar(
            out=dst_m, in0=e1[:], scalar1=1.0, scalar2=-1.0,
            op0=mybir.AluOpType.min, op1=mybir.AluOpType.add,
        )

    def elu_chunk(z_ps, parts, dst_a, dst_m, cidx):
        e1 = work.tile(
            [parts, 2, 512], bf16, tag=f"e1{parts}", name=f"e1_{cidx}_{r}", bufs=3
        )
        nc.scalar.activation(e1[:], z_ps[:], AF.Exp)
        if CFG["elu_max"] == "act" or (CFG["elu_max"] == "alt" and cidx & 1):
            nc.scalar.activation(dst_a, z_ps[:], AF.Relu)
        else:
            nc.vector.tensor_scalar_max(dst_a, z_ps[:], 0.0)
        mn = CFG["elu_min"]
        eng_min = nc.gpsimd if (mn == "pool" or (mn == "alt" and cidx & 1)) else nc.vector
        eng_min.tensor_scalar(
            out=dst_m, in0=e1[:], scalar1=1.0, scalar2=-1.0,
            op0=mybir.AluOpType.min, op1=mybir.AluOpType.add,
        )
        if CFG["hc_add"] != "none":
            eng_add = nc.gpsimd if CFG["hc_add"] == "pool" else nc.vector
            eng_add.tensor_tensor(
                out=dst_a, in0=dst_a, in1=dst_m, op=mybir.AluOpType.add
            )

    ho_ps = psum.tile([128, NT, NCLASS], f32, tag="acc", name=f"ho_{r}")

    sqo = const.tile([128, NT, NCLASS], bf16, name=f"sqo{r}")
    qo = const.tile([128, NT], f32, name=f"qo{r}")
    rno = const.tile([128, NT], bf16, name=f"rno{r}")
    h1oa = const.tile([128, NT, NCLASS], bf16, name=f"h1oa{r}")
    go_t = psum.tile([NCLASS, NCLASS], f32, tag="accsm", name=f"goacc_{r}")
    go_ps = go_t[:]
    h1ot = const.tile([NCLASS, N], bf16, name=f"h1ot{r}")

    def stage2_half(half):
        nc.scalar.activation(
            sqo[:, ts(half, 16), :], ho_ps[:, ts(half, 16), :], AF.Square
        )
        nc.vector.reduce_sum(
            qo[:, ts(half, 16)], sqo[:, ts(half, 16), :], axis=mybir.AxisListType.X
        )
        if CFG["rsqrt"] == "abs":
            nc.scalar.activation(
                rno[:, ts(half, 16)], qo[:, ts(half, 16)],
                mybir.ActivationFunctionType.Abs_reciprocal_sqrt,
            )
        else:
            qoi = small.tile([128, 16], f32, tag="qoi", name=f"qoi{half}_{r}")
            nc.vector.reciprocal(qoi[:], qo[:, ts(half, 16)])
            nc.scalar.sqrt(rno[:, ts(half, 16)], qoi[:])
        nc.vector.tensor_mul(
            h1oa[:, ts(half, 16), :],
            ho_ps[:, ts(half, 16), :],
            bcast_free(rno[:, ts(half, 16)], NCLASS),
        )
        for gp in range(2 * half, 2 * half + 2):
            tro = psum.tile(
                [NCLASS, 8, 128], bf16, tag="z", name=f"tro{gp}_{r}", bufs=2
            )
            for j in range(8):
                t = 8 * gp + j
                nc.tensor.transpose(tro[:, j, :], h1oa[:, t, :], id128[:])
                nc.tensor.matmul(
                    go_ps, h1oa[:, t, :], h1oa[:, t, :],
                    start=(t == 0), stop=(t == NT - 1),
                )
            drain_copy(
                h1ot[:, ts(gp, 1024)].rearrange("p (a b) -> p a b", a=8), tro[:]
            )

    for c in range(4):
        zhi = psum.tile([128, 2, 512], f32, tag="wide", name=f"zhi{c}_{r}", bufs=2)
        for i in range(2):
            nc.tensor.matmul(
                zhi[:, i, :], mblk_b[:], h1t01[:, ts(2 * c + i, 512)],
                start=True, stop=True,
            )
        elu_chunk(
            zhi, 128,
            hca_hi[:, ts(c, 1024)].rearrange("p (a b) -> p a b", a=2),
            hcm_hi[:, ts(c, 1024)].rearrange("p (a b) -> p a b", a=2),
            2 * c,
        )
        zlo = psum.tile([NHID, 2, 512], f32, tag="wide", name=f"zlo{c}_{r}", bufs=2)
        for i in range(2):
            nc.tensor.matmul(
                zlo[:, i, :], m2_b[:], h1t2[:, ts(2 * c + i, 512)],
                start=True, stop=True,
            )
        elu_chunk(
            zlo, NHID,
            hca_lo[:, ts(c, 1024)].rearrange("p (a b) -> p a b", a=2),
            hcm_lo[:, ts(c, 1024)].rearrange("p (a b) -> p a b", a=2),
            2 * c + 1,
        )
        for t in range(8 * c, 8 * c + 8):
            nc.tensor.matmul(
                ho_ps[:, t, :], hca_hi[:, ts(t, 128)], wot_hi_sb[:],
                start=True, stop=False,
            )
            nc.tensor.matmul(
                ho_ps[:, t, :], hcm_hi[:, ts(t, 128)], wot_hi_sb[:],
                start=False, stop=False,
            )
            nc.tensor.matmul(
                ho_ps[:, t, :], hca_lo[:, ts(t, 128)], wot_lo_sb[:],
                start=False, stop=False,
            )
            nc.tensor.matmul(
                ho_ps[:, t, :], hcm_lo[:, ts(t, 128)], wot_lo_sb[:],
                start=False, stop=not with_bias_o,
            )
            if with_bias_o:
                nc.tensor.matmul(
                    ho_ps[:, t, :], ones1[:], bo_sb[:], start=False, stop=True
                )

    # (ho accumulated inside the elu loop above)
    stage2_half(0)
    stage2_half(1)
    go_b = const.tile([NCLASS, NCLASS], bf16, name=f"gob{r}")
    nc.vector.tensor_copy(go_b[:], go_ps)

    to_ps = psum.tile([128, NT, NCLASS], f32, tag="acc", name=f"to_{r}")
    scro = const.tile([128, NT, NCLASS], bf16, name=f"scro{r}")
    qo2 = const.tile([128, NT], f32, name=f"qo2{r}")
    icno = const.tile([128, NT], bf16, name=f"icno{r}")
    h1so = const.tile([128, NT, NCLASS], bf16, name=f"h1so{r}")
    mo_t = psum.tile([NCLASS, NCLASS], f32, tag="accsm", name=f"moacc_{r}")
    mo_ps = mo_t[:]
    for half in range(2):
        for t in range(16 * half, 16 * half + 16):
            nc.tensor.matmul(
                to_ps[:, t, :], h1ot[:, ts(t, 128)], go_b[:], start=True, stop=True
            )
        nc.vector.tensor_mul(
            scro[:, ts(half, 16), :], to_ps[:, ts(half, 16), :],
            h1oa[:, ts(half, 16), :],
        )
        nc.vector.reduce_sum(
            qo2[:, ts(half, 16)], scro[:, ts(half, 16), :],
            axis=mybir.AxisListType.X,
        )
        if CFG["rsqrt"] == "abs":
            nc.scalar.activation(
                icno[:, ts(half, 16)], qo2[:, ts(half, 16)],
                mybir.ActivationFunctionType.Abs_reciprocal_sqrt,
            )
        else:
            qo2i = small.tile([128, 16], f32, tag="qo2i", name=f"qo2i{half}_{r}")
            nc.vector.reciprocal(qo2i[:], qo2[:, ts(half, 16)])
            nc.scalar.sqrt(icno[:, ts(half, 16)], qo2i[:])
        nc.vector.tensor_mul(
            h1so[:, ts(half, 16), :], h1oa[:, ts(half, 16), :],
            bcast_free(icno[:, ts(half, 16)], NCLASS),
        )
        for t in range(16 * half, 16 * half + 16):
            nc.tensor.matmul(
                mo_ps, h1oa[:, t, :], h1so[:, t, :],
                start=(t == 0), stop=(t == NT - 1),
            )
    mo_b = const.tile([NCLASS, NCLASS], bf16, name=f"mob{r}")
    nc.vector.tensor_copy(mo_b[:], mo_ps)

    fo_ps = psum.tile([128, NT, NCLASS], f32, tag="acc", name=f"fo_{r}")
    for t in range(NT):
        nc.tensor.matmul(
            fo_ps[:, t, :], h1ot[:, ts(t, 128)], mo_b[:], start=True, stop=True
        )
    fot_sb = const.tile([128, NT * NCLASS], f32, name=f"fot{r}")
    drain_copy(fot_sb[:, 0:256].rearrange("p (a b) -> p a b", a=16), fo_ps[:, 0:16, :])
    nc.sync.dma_start(out=tensors["outt"][:, 0:256], in_=fot_sb[:, 0:256])
    drain_copy(fot_sb[:, 256:512].rearrange("p (a b) -> p a b", a=16), fo_ps[:, 16:32, :])
    nc.sync.dma_start(out=tensors["outt"][:, 256:512], in_=fot_sb[:, 256:512])


def build_program(reps=1, mode="rep", with_bias=(False, False), loop=1):
    key = (reps, mode, with_bias, loop, tuple(sorted(CFG.items())))
    if key in _prog_cache:
        return _prog_cache[key]

    _patch_tile_drain()
    import concourse.bass as bass
    import concourse.tile as tile
    import concourse.mybir as mybir
    from contextlib import ExitStack

    f32 = mybir.dt.float32
    bf16 = mybir.dt.float16
    nc = bass.Bass(num_devices=N_CORES)
    tensors = {
        "xb": nc.dram_tensor("xb", [128, N], bf16, kind="ExternalInput"),
        "w123t": nc.dram_tensor("w123t", [128, D3], bf16, kind="ExternalInput"),
        "wot_hi": nc.dram_tensor("wot_hi", [128, NCLASS], bf16, kind="ExternalInput"),
        "wot_lo": nc.dram_tensor("wot_lo", [128, NCLASS], bf16, kind="ExternalInput"),
        "b123": nc.dram_tensor("b123", [1, D3], bf16, kind="ExternalInput"),
        "bo16": nc.dram_tensor("bo16", [1, NCLASS], bf16, kind="ExternalInput"),
        "outt": nc.dram_tensor("outt", [128, NT * NCLASS], f32, kind="ExternalOutput"),
    }

    with tile.TileContext(nc) as tc:
        if loop > 1:
            with tc.For_i(0, loop, 1):
                for r in range(reps):
                    with ExitStack() as ctx:
                        _emit_body(nc, tc, ctx, tensors, r, with_bias)
        else:
            for r in range(reps):
                with ExitStack() as ctx:
                    _emit_body(nc, tc, ctx, tensors, r, with_bias)

    _split_multi_waits(nc)
    _prog_cache[key] = nc
    return nc


def make_in_maps(x, W1, b1, W2, b2, W3, b3, Wo, bo, mode="rep"):
    bf = np.float16
    x_mem = np.asarray(x, dtype=np.float32).reshape(NFEAT, N)
    w123t = np.ascontiguousarray(
        np.concatenate(
            [np.asarray(W1).T, np.asarray(W2).T, np.asarray(W3).T], axis=1
        ),
        dtype=np.float32,
    )
    wot = np.ascontiguousarray(np.asarray(Wo).T, dtype=np.float32)
    b123 = (
        np.concatenate([np.asarray(b1), np.asarray(b2), np.asarray(b3)])
        .reshape(1, D3)
        .astype(bf)
    )
    common = {
        "xb": x_mem.astype(bf),
        "w123t": w123t.astype(bf),
        "wot_hi": np.ascontiguousarray(wot[:128]).astype(bf),
        "wot_lo": np.ascontiguousarray(np.concatenate([wot[128:], wot[128:]], axis=0)).astype(bf),
        "b123": b123,
        "bo16": np.asarray(bo).reshape(1, NCLASS).astype(bf),
    }
    return [dict(common) for _ in range(N_CORES)]


def assemble_output(results, mode="rep"):
    fot = np.asarray(results[0]["outt"], dtype=np.float32)
    h = fot.reshape(128, NT, NCLASS).transpose(1, 0, 2).reshape(N, NCLASS)
    return np.ascontiguousarray(h.T.reshape(1, NCLASS, 64, 64), dtype=np.float32)


def kernel(x, W1, b1, W2, b2, W3, b3, Wo, bo):
    from concourse.bass_utils import run_bass_kernel_spmd

    with_bias = (
        any(np.any(np.asarray(b)) for b in (b1, b2, b3)),
        bool(np.any(np.asarray(bo))),
    )
    nc = build_program(reps=1, mode="rep", with_bias=with_bias)
    in_maps = make_in_maps(x, W1, b1, W2, b2, W3, b3, Wo, bo)
    res = run_bass_kernel_spmd(nc, in_maps, list(range(N_CORES)))
    return assemble_output(res.results)
